# revision 3
# baseline (speedup 1.0000x reference)
"""LiquidMoE Trainium2 kernel: expert-parallel across 8 NeuronCores, v2.

Strategy: 16 experts sharded 2-per-core across 8 cores, with *balanced*
slot capacities: experts are sorted by routed-token count; the top-8 go in
slot 0 (capacity CAP0 = max count of that group, rounded) and the bottom-8
in slot 1 (CAP1 likewise). Every core does CAP0+CAP1 (~4208) columns
instead of v1's fixed 2*2176=4352. Host computes the (tiny) gate + top-k
routing and packs per-expert token matrices; each core runs the heavy
expert FFN (x@W1 -> gelu -> @W2) for its 2 experts in bf16 on the
TensorEngine; host applies combine weights and scatter-adds.

Device layout: everything is computed transposed (tokens on the matmul free
axis) so no on-device transposes are needed:
  H^T[f, t] = gelu(W1[d,f].T @ X^T[d, t] + b1)   (lhsT = W1 tile, rhs = X^T)
  Y^T[d, t] = W2[f,d].T @ H^T[f, t]              (lhsT = W2 tile, rhs = H^T)

v2 vs v1 (trace-driven):
  - balanced dynamic CAP0/CAP1 (saves ~144 padded columns/core of matmul)
  - consolidated DMAs: xb is ONE dma_start per chunk ([128,8,tw]), W1 is 4
    slice-DMAs ([128,8,1024]) and W2 is 8 slice-DMAs ([128,4,1024]) per
    expert. v1 spent ~0.6us of sync-engine issue time per dma_start, which
    serialized the cold-start W1 stream (~35us of PE idle at startup).
  - b1 (gelu bias) DMA issued before W1 so the first gelu doesn't hold
    PSUM recycling hostage (v1 lost ~19us to this).
  - yT emitted as bf16 (halves output DMA, shortens drain).
"""

import sys
import numpy as np

B, S, D, E, F, TOPK = 2, 4096, 1024, 16, 4096, 4
T = B * S
N_CORES = 8
EPC = E // N_CORES          # experts per core (2 slots)
ROUND = 8                   # column-capacity rounding

_NC_CACHE = {}
_LAST_RESULT = None  # BassKernelResults of the most recent device run


def _chunk_plan(cap, first_small=False):
    """Split cap into chunks of <=512, as few and as even as possible,
    remainder-ish chunk last. With first_small, the first chunk is 256 wide
    so the cold-start needs less DMA before compute."""
    plan = []
    if first_small:
        plan.append(256)
        cap -= 256
    k = -(-cap // 512)
    base = -(-(cap // k) // 8) * 8        # near-even, multiple of 8
    plan += [base] * (k - 1) + [cap - base * (k - 1)]
    assert all(0 < c <= 512 for c in plan) and sum(plan) == cap + (
        256 if first_small else 0), plan
    return plan


def build_nc(caps, d=D, f=F, n_cores=N_CORES):
    import concourse.mybir as mybir
    import concourse.tile as tile
    from concourse import bacc

    dt = mybir.dt
    DB, FB = d // 128, f // 128   # number of 128-blocks in d and f
    caps = list(caps)
    nslots = len(caps)
    plans = [_chunk_plan(c, first_small=(s == 0))
             for s, c in enumerate(caps)]
    offs = [0]                    # column offset of each slot in xT/yT
    for c in caps[:-1]:
        offs.append(offs[-1] + c)
    ncols = sum(caps)

    # W1 f-slices: small first slice so the cold-start's first matmul
    # group only needs 0.5MB of W1 in flight, then full-width slices.
    W1W = [256, 768, 1024, 1024, 1024]
    W1OFF = [0]
    for w in W1W[:-1]:
        W1OFF.append(W1OFF[-1] + w)
    W2SL = 8                      # W2 f-slices per expert
    W2SR = FB // W2SL             # 4 f0-groups per W2 slice

    nc = bacc.Bacc("TRN2", target_bir_lowering=False, debug=False,
                   num_devices=n_cores)
    xT = nc.dram_tensor("xT", [d, ncols], dt.bfloat16, kind="ExternalInput")
    w1 = nc.dram_tensor("w1", [nslots, d, f], dt.bfloat16, kind="ExternalInput")
    b1 = nc.dram_tensor("b1", [nslots, f], dt.float32, kind="ExternalInput")
    w2 = nc.dram_tensor("w2", [nslots, f, d], dt.bfloat16, kind="ExternalInput")
    yT = nc.dram_tensor("yT", [d, ncols], dt.bfloat16, kind="ExternalOutput")

    with tile.TileContext(nc) as tc:
        with (
            tc.tile_pool(name="pw0", bufs=2) as pw0,
            tc.tile_pool(name="pwA", bufs=1) as pwA,
            tc.tile_pool(name="pw1", bufs=4) as pw1,
            tc.tile_pool(name="pw2", bufs=W2SL) as pw2,
            tc.tile_pool(name="phb", bufs=FB + 1) as phb,
            tc.tile_pool(name="pxb", bufs=2) as pxb,
            tc.tile_pool(name="pout", bufs=6) as pout,
            tc.tile_pool(name="pb1", bufs=2) as pb1,
            tc.tile_pool(name="ps1", bufs=3, space="PSUM") as ps1,
            tc.tile_pool(name="ps2", bufs=4, space="PSUM") as ps2,
        ):
            w1pools = [pw0, pwA, pw1, pw1, pw1]
            def issue_xb(e, t0, tw):
                xt = pxb.tile([128, DB, tw], dt.bfloat16, tag="xb", name="xb")
                nc.sync.dma_start(
                    xt[:],
                    xT.ap()[:, t0:t0 + tw].rearrange("(a p) t -> p a t",
                                                     p=128))
                return xt

            # slot 0 first (cold-start small chunk); end on the slot with
            # the smallest final chunk so the drain is short.
            slot_order = [0] + sorted(range(1, nslots),
                                      key=lambda s: -plans[s][-1])
            for e in slot_order:
                chunks = plans[e]
                starts = [offs[e]]
                for tw in chunks[:-1]:
                    starts.append(starts[-1] + tw)

                # ---- b1 first: tiny, and the first gelu needs it.
                # [128, FB] (partition = f within block, free = f0)
                b1t = pb1.tile([128, FB], dt.float32, tag="b1t", name="b1t")
                nc.sync.dma_start(
                    b1t[:], b1.ap()[e].rearrange("(a b) -> b a", b=128))

                # first chunk's activations so mm1 can start ASAP
                xb_pending = {0: issue_xb(e, starts[0], chunks[0])}

                # ---- W1 bf16 resident, slices of [128, DB, w]
                # (slice-major arrival: mm1 f0-groups chase the stream)
                w1s = []
                for fs, w in enumerate(W1W):
                    t1 = w1pools[fs].tile([128, DB, w], dt.bfloat16,
                                          tag=f"w1p{min(fs, 2)}", name="w1s")
                    nc.sync.dma_start(
                        t1[:],
                        w1.ap()[e, :, W1OFF[fs]:W1OFF[fs] + w].rearrange(
                            "(a p) f -> p a f", p=128))
                    w1s.append(t1)

                # W2 slice tiles allocated here; DMAs issued interleaved
                # into chunk 0's mm1 below (needed from mm2 of chunk 0 on).
                w2s = [pw2.tile([128, W2SR, d], dt.bfloat16, tag="w2s",
                                name="w2s")
                       for _ in range(W2SL)]

                # ---- main token-chunk loop
                for ci, tw in enumerate(chunks):
                    tsl = slice(starts[ci], starts[ci] + tw)
                    xb = xb_pending.pop(ci)
                    if ci + 1 < len(chunks):
                        xb_pending[ci + 1] = issue_xb(
                            e, starts[ci + 1], chunks[ci + 1])

                    # mm1 + gelu: H^T[f0] = gelu(W1.T @ X^T + b1)
                    hb = []
                    for f0 in range(FB):
                        if ci == 0 and f0 % W2SR == 0:
                            g = f0 // W2SR
                            nc.sync.dma_start(
                                w2s[g][:],
                                w2.ap()[e, g * W2SR * 128:
                                        (g + 1) * W2SR * 128, :].rearrange(
                                    "(a p) dc -> p a dc", p=128))
                        ps = ps1.tile([128, tw], dt.float32, tag="ps1",
                                      name="ps1t")
                        fpos = f0 * 128
                        fs = max(i for i, o in enumerate(W1OFF) if o <= fpos)
                        fo = fpos - W1OFF[fs]
                        for d0 in range(DB):
                            nc.tensor.matmul(
                                ps[:],
                                w1s[fs][:, d0, fo:fo + 128],
                                xb[:, d0, :],
                                start=(d0 == 0), stop=(d0 == DB - 1))
                        ht = phb.tile([128, tw], dt.bfloat16, tag="hb",
                                      name="hb")
                        nc.scalar.activation(
                            ht[:], ps[:], mybir.ActivationFunctionType.Gelu,
                            bias=b1t[:, f0:f0 + 1])
                        hb.append(ht)

                    # mm2: Y^T[dd0] = W2.T @ H^T
                    for dd0 in range(DB):
                        ps_o = ps2.tile([128, tw], dt.float32, tag="ps2",
                                        name="ps2t")
                        for f0 in range(FB):
                            g, j = divmod(f0, W2SR)
                            nc.tensor.matmul(
                                ps_o[:],
                                w2s[g][:, j, dd0 * 128:(dd0 + 1) * 128],
                                hb[f0][:],
                                start=(f0 == 0), stop=(f0 == FB - 1))
                        ot = pout.tile([128, tw], dt.bfloat16, tag="ot",
                                       name="ot")
                        nc.vector.tensor_copy(ot[:], ps_o[:])
                        nc.sync.dma_start(
                            yT.ap()[dd0 * 128:(dd0 + 1) * 128, tsl], ot[:])

    nc.compile()
    return nc


def _route(x, gate_w, trust_scores):
    """Host routing: gates, trust-weighted top-k, softmax. float64 for
    numerics close to the fp32 reference."""
    xf = np.asarray(x, np.float32).reshape(-1, D)
    g = xf.astype(np.float64) @ np.asarray(gate_w, np.float64).T
    tw = g * (1.0 / (1.0 + np.exp(-np.asarray(trust_scores, np.float64))))
    order = np.argsort(-tw, axis=-1, kind="stable")[:, :TOPK]      # [T, K]
    vals = np.take_along_axis(tw, order, axis=-1)
    vals = vals - vals.max(-1, keepdims=True)
    p = np.exp(vals)
    probs = (p / p.sum(-1, keepdims=True)).astype(np.float32)       # [T, K]
    return xf, order, probs


def kernel(x, gate_w, trust_scores, w1, b1, w2, b2):
    import ml_dtypes
    bf16 = ml_dtypes.bfloat16

    xf, order, probs = _route(x, gate_w, trust_scores)

    # per-expert token index lists + combine weights
    tok_idx, wgt = [], []
    for e in range(E):
        sel = np.nonzero((order == e).any(-1))[0]
        ke = (order[sel] == e).argmax(-1)
        tok_idx.append(sel)
        wgt.append(probs[sel, ke])

    # Split each expert's token set into two halves; 32 pieces total.
    # Sort pieces by length desc; slot j (of 4) takes ranks [8j, 8j+8),
    # one piece per core. Every core runs sum(caps) columns.
    NSLOTS = 4
    pieces = []                           # (len, expert, sel_slice, wgt_slice)
    for e in range(E):
        n = len(tok_idx[e])
        h = (n + 1) // 2
        pieces.append((h, e, tok_idx[e][:h], wgt[e][:h]))
        pieces.append((n - h, e, tok_idx[e][h:], wgt[e][h:]))
    pieces.sort(key=lambda p: -p[0])

    def rup(v):
        return int(-(-v // ROUND) * ROUND)
    caps = tuple(max(rup(max(p[0] for p in pieces[8 * j:8 * j + 8])),
                     264 if j == 0 else 8)
                 for j in range(NSLOTS))
    offs = [0]
    for c in caps[:-1]:
        offs.append(offs[-1] + c)

    if caps not in _NC_CACHE:
        _NC_CACHE[caps] = build_nc(caps)
    nc = _NC_CACHE[caps]

    w1n = np.ascontiguousarray(np.asarray(w1, np.float32).astype(bf16))
    b1n = np.ascontiguousarray(np.asarray(b1, np.float32))
    w2n = np.ascontiguousarray(np.asarray(w2, np.float32).astype(bf16))
    b2n = np.asarray(b2, np.float32)

    ncols = sum(caps)
    in_maps = []
    core_pieces = []                              # [core][slot] -> piece
    for c in range(N_CORES):
        slots = [pieces[8 * j + c] for j in range(NSLOTS)]
        xT = np.zeros((D, ncols), bf16)
        for j, (n, e, sel, _) in enumerate(slots):
            xT[:, offs[j]:offs[j] + n] = xf[sel].T.astype(bf16)
        es = [p[1] for p in slots]
        in_maps.append({
            "xT": xT,
            "w1": np.ascontiguousarray(w1n[es]),
            "b1": np.ascontiguousarray(b1n[es]),
            "w2": np.ascontiguousarray(w2n[es]),
        })
        core_pieces.append(slots)

    from concourse.bass_utils import run_bass_kernel_spmd
    res = run_bass_kernel_spmd(nc, in_maps, list(range(N_CORES)))
    global _LAST_RESULT
    _LAST_RESULT = res

    out = np.zeros_like(xf)
    for c in range(N_CORES):
        yT = res.results[c]["yT"]                    # [D, ncols] bf16
        for j, (n, e, sel, w) in enumerate(core_pieces[c]):
            if n == 0:
                continue
            y = yT[:, offs[j]:offs[j] + n].T.astype(np.float32) + b2n[e]
            out[sel] += w[:, None] * y
    return out.reshape(B, S, D)


# revision 4
# speedup vs baseline: 1.0122x; 1.0122x over previous
"""LiquidMoE Trainium2 kernel: expert-parallel across 8 NeuronCores.

Host computes the (tiny) gate + trust-weighted top-k routing; the device
runs the heavy per-expert FFN (x @ W1 -> gelu -> @ W2) in bf16 on the
TensorEngine; host applies combine weights + b2 and scatter-adds.

Load balancing (SPMD requires an identical program on all 8 cores): each
expert's routed-token set is split into two halves -> 32 pieces. Pieces
are sorted by length; slot j of 4 takes ranks [8j, 8j+8), one piece per
core, and slot capacity = max piece length in the slot (rounded to 8).
Every core computes sum(caps) (~4152) token-columns vs 4096 for a perfect
split and 4352 for the original fixed 2-experts-per-core layout. Weights
for one expert stream to (at most) two cores; weight DMA is fully hidden
under compute.

Device layout: everything is computed transposed (tokens on the matmul
free axis) so no on-device transposes are needed:
  H^T[f, t] = gelu(W1[d,f].T @ X^T[d, t] + b1)   (lhsT = W1 tile, rhs = X^T)
  Y^T[d, t] = W2[f,d].T @ H^T[f, t]              (lhsT = W2 tile, rhs = H^T)

Trace-driven details:
  - consolidated DMAs: xb is ONE dma_start per chunk ([128,8,tw]); W1 is 5
    slice-DMAs (first slice small so the first matmul group needs only
    0.5MB in flight) and W2 is 8 slice-DMAs per expert. Each dma_start
    costs ~0.6us of sync-engine issue time, which serialized the cold
    start (~35us of PE idle originally).
  - b1 (gelu bias) DMA issued before W1 so the first gelu doesn't hold
    PSUM recycling hostage.
  - first chunk of the first slot is 256 wide: less DMA on the cold-start
    critical path before the PE can begin.
  - chunks are near-even splits <=512 wide (a tiny chunk would expose the
    per-matmul LDWEIGHTS latency); the program ends on the slot with the
    smallest final chunk to shorten the drain.
  - yT emitted as bf16 (halves output DMA; rel err stays ~3.8e-3).
Measured: 924880 ns (baseline 986947 ns), l2 rel err 3.76e-3.
"""

import sys
import numpy as np

B, S, D, E, F, TOPK = 2, 4096, 1024, 16, 4096, 4
T = B * S
N_CORES = 8
EPC = E // N_CORES          # experts per core (2 slots)
ROUND = 8                   # column-capacity rounding

_NC_CACHE = {}
_LAST_RESULT = None  # BassKernelResults of the most recent device run


def _chunk_plan(cap, first_small=False):
    """Split cap into chunks of <=512, as few and as even as possible,
    remainder-ish chunk last. With first_small, the first chunk is 256 wide
    so the cold-start needs less DMA before compute."""
    plan = []
    if first_small:
        plan.append(256)
        cap -= 256
    k = -(-cap // 512)
    base = -(-(cap // k) // 8) * 8        # near-even, multiple of 8
    plan += [base] * (k - 1) + [cap - base * (k - 1)]
    assert all(0 < c <= 512 for c in plan) and sum(plan) == cap + (
        256 if first_small else 0), plan
    return plan


def build_nc(caps, d=D, f=F, n_cores=N_CORES):
    import concourse.mybir as mybir
    import concourse.tile as tile
    from concourse import bacc

    dt = mybir.dt
    DB, FB = d // 128, f // 128   # number of 128-blocks in d and f
    caps = list(caps)
    nslots = len(caps)
    plans = [_chunk_plan(c, first_small=(s == 0))
             for s, c in enumerate(caps)]
    offs = [0]                    # column offset of each slot in xT/yT
    for c in caps[:-1]:
        offs.append(offs[-1] + c)
    ncols = sum(caps)

    # W1 f-slices: small first slice so the cold-start's first matmul
    # group only needs 0.5MB of W1 in flight, then full-width slices.
    W1W = [256, 768, 1024, 1024, 1024]
    W1OFF = [0]
    for w in W1W[:-1]:
        W1OFF.append(W1OFF[-1] + w)
    W2SL = 8                      # W2 f-slices per expert
    W2SR = FB // W2SL             # 4 f0-groups per W2 slice

    nc = bacc.Bacc("TRN2", target_bir_lowering=False, debug=False,
                   num_devices=n_cores)
    xT = nc.dram_tensor("xT", [d, ncols], dt.bfloat16, kind="ExternalInput")
    w1 = nc.dram_tensor("w1", [nslots, d, f], dt.bfloat16, kind="ExternalInput")
    b1 = nc.dram_tensor("b1", [nslots, f], dt.float32, kind="ExternalInput")
    w2 = nc.dram_tensor("w2", [nslots, f, d], dt.bfloat16, kind="ExternalInput")
    yT = nc.dram_tensor("yT", [d, ncols], dt.bfloat16, kind="ExternalOutput")

    with tile.TileContext(nc) as tc:
        with (
            tc.tile_pool(name="pw0", bufs=2) as pw0,
            tc.tile_pool(name="pwA", bufs=1) as pwA,
            tc.tile_pool(name="pw1", bufs=4) as pw1,
            tc.tile_pool(name="pw2", bufs=W2SL) as pw2,
            tc.tile_pool(name="phb", bufs=FB + 1) as phb,
            tc.tile_pool(name="pxb", bufs=2) as pxb,
            tc.tile_pool(name="pout", bufs=6) as pout,
            tc.tile_pool(name="pb1", bufs=2) as pb1,
            tc.tile_pool(name="ps1", bufs=3, space="PSUM") as ps1,
            tc.tile_pool(name="ps2", bufs=4, space="PSUM") as ps2,
        ):
            w1pools = [pw0, pwA, pw1, pw1, pw1]
            def issue_xb(e, t0, tw):
                xt = pxb.tile([128, DB, tw], dt.bfloat16, tag="xb", name="xb")
                nc.sync.dma_start(
                    xt[:],
                    xT.ap()[:, t0:t0 + tw].rearrange("(a p) t -> p a t",
                                                     p=128))
                return xt

            # slot 0 first (cold-start small chunk); end on the slot with
            # the smallest final chunk so the drain is short.
            slot_order = [0] + sorted(range(1, nslots),
                                      key=lambda s: -plans[s][-1])
            for e in slot_order:
                chunks = plans[e]
                starts = [offs[e]]
                for tw in chunks[:-1]:
                    starts.append(starts[-1] + tw)

                # ---- b1 first: tiny, and the first gelu needs it.
                # [128, FB] (partition = f within block, free = f0)
                b1t = pb1.tile([128, FB], dt.float32, tag="b1t", name="b1t")
                nc.sync.dma_start(
                    b1t[:], b1.ap()[e].rearrange("(a b) -> b a", b=128))

                # first chunk's activations so mm1 can start ASAP
                xb_pending = {0: issue_xb(e, starts[0], chunks[0])}

                # ---- W1 bf16 resident, slices of [128, DB, w]
                # (slice-major arrival: mm1 f0-groups chase the stream)
                w1s = []
                for fs, w in enumerate(W1W):
                    t1 = w1pools[fs].tile([128, DB, w], dt.bfloat16,
                                          tag=f"w1p{min(fs, 2)}", name="w1s")
                    nc.sync.dma_start(
                        t1[:],
                        w1.ap()[e, :, W1OFF[fs]:W1OFF[fs] + w].rearrange(
                            "(a p) f -> p a f", p=128))
                    w1s.append(t1)

                # W2 slice tiles allocated here; DMAs issued interleaved
                # into chunk 0's mm1 below (needed from mm2 of chunk 0 on).
                w2s = [pw2.tile([128, W2SR, d], dt.bfloat16, tag="w2s",
                                name="w2s")
                       for _ in range(W2SL)]

                # ---- main token-chunk loop
                for ci, tw in enumerate(chunks):
                    tsl = slice(starts[ci], starts[ci] + tw)
                    xb = xb_pending.pop(ci)
                    if ci + 1 < len(chunks):
                        xb_pending[ci + 1] = issue_xb(
                            e, starts[ci + 1], chunks[ci + 1])

                    # mm1 + gelu: H^T[f0] = gelu(W1.T @ X^T + b1)
                    hb = []
                    for f0 in range(FB):
                        if ci == 0 and f0 % W2SR == 0:
                            g = f0 // W2SR
                            nc.sync.dma_start(
                                w2s[g][:],
                                w2.ap()[e, g * W2SR * 128:
                                        (g + 1) * W2SR * 128, :].rearrange(
                                    "(a p) dc -> p a dc", p=128))
                        ps = ps1.tile([128, tw], dt.float32, tag="ps1",
                                      name="ps1t")
                        fpos = f0 * 128
                        fs = max(i for i, o in enumerate(W1OFF) if o <= fpos)
                        fo = fpos - W1OFF[fs]
                        for d0 in range(DB):
                            nc.tensor.matmul(
                                ps[:],
                                w1s[fs][:, d0, fo:fo + 128],
                                xb[:, d0, :],
                                start=(d0 == 0), stop=(d0 == DB - 1))
                        ht = phb.tile([128, tw], dt.bfloat16, tag="hb",
                                      name="hb")
                        nc.scalar.activation(
                            ht[:], ps[:], mybir.ActivationFunctionType.Gelu,
                            bias=b1t[:, f0:f0 + 1])
                        hb.append(ht)

                    # mm2: Y^T[dd0] = W2.T @ H^T
                    for dd0 in range(DB):
                        ps_o = ps2.tile([128, tw], dt.float32, tag="ps2",
                                        name="ps2t")
                        for f0 in range(FB):
                            g, j = divmod(f0, W2SR)
                            nc.tensor.matmul(
                                ps_o[:],
                                w2s[g][:, j, dd0 * 128:(dd0 + 1) * 128],
                                hb[f0][:],
                                start=(f0 == 0), stop=(f0 == FB - 1))
                        ot = pout.tile([128, tw], dt.bfloat16, tag="ot",
                                       name="ot")
                        nc.vector.tensor_copy(ot[:], ps_o[:])
                        nc.sync.dma_start(
                            yT.ap()[dd0 * 128:(dd0 + 1) * 128, tsl], ot[:])

    nc.compile()
    return nc


def _route(x, gate_w, trust_scores):
    """Host routing: gates, trust-weighted top-k, softmax. float64 for
    numerics close to the fp32 reference."""
    xf = np.asarray(x, np.float32).reshape(-1, D)
    g = xf.astype(np.float64) @ np.asarray(gate_w, np.float64).T
    tw = g * (1.0 / (1.0 + np.exp(-np.asarray(trust_scores, np.float64))))
    order = np.argsort(-tw, axis=-1, kind="stable")[:, :TOPK]      # [T, K]
    vals = np.take_along_axis(tw, order, axis=-1)
    vals = vals - vals.max(-1, keepdims=True)
    p = np.exp(vals)
    probs = (p / p.sum(-1, keepdims=True)).astype(np.float32)       # [T, K]
    return xf, order, probs


def kernel(x, gate_w, trust_scores, w1, b1, w2, b2):
    import ml_dtypes
    bf16 = ml_dtypes.bfloat16

    xf, order, probs = _route(x, gate_w, trust_scores)

    # per-expert token index lists + combine weights
    tok_idx, wgt = [], []
    for e in range(E):
        sel = np.nonzero((order == e).any(-1))[0]
        ke = (order[sel] == e).argmax(-1)
        tok_idx.append(sel)
        wgt.append(probs[sel, ke])

    # Split each expert's token set into two halves; 32 pieces total.
    # Sort pieces by length desc; slot j (of 4) takes ranks [8j, 8j+8),
    # one piece per core. Every core runs sum(caps) columns.
    NSLOTS = 4
    pieces = []                           # (len, expert, sel_slice, wgt_slice)
    for e in range(E):
        n = len(tok_idx[e])
        h = (n + 1) // 2
        pieces.append((h, e, tok_idx[e][:h], wgt[e][:h]))
        pieces.append((n - h, e, tok_idx[e][h:], wgt[e][h:]))
    pieces.sort(key=lambda p: -p[0])

    def rup(v):
        return int(-(-v // ROUND) * ROUND)
    caps = tuple(max(rup(max(p[0] for p in pieces[8 * j:8 * j + 8])),
                     264 if j == 0 else 8)
                 for j in range(NSLOTS))
    offs = [0]
    for c in caps[:-1]:
        offs.append(offs[-1] + c)

    if caps not in _NC_CACHE:
        _NC_CACHE[caps] = build_nc(caps)
    nc = _NC_CACHE[caps]

    w1n = np.ascontiguousarray(np.asarray(w1, np.float32).astype(bf16))
    b1n = np.ascontiguousarray(np.asarray(b1, np.float32))
    w2n = np.ascontiguousarray(np.asarray(w2, np.float32).astype(bf16))
    b2n = np.asarray(b2, np.float32)

    ncols = sum(caps)
    in_maps = []
    core_pieces = []                              # [core][slot] -> piece
    for c in range(N_CORES):
        slots = [pieces[8 * j + c] for j in range(NSLOTS)]
        xT = np.zeros((D, ncols), bf16)
        for j, (n, e, sel, _) in enumerate(slots):
            xT[:, offs[j]:offs[j] + n] = xf[sel].T.astype(bf16)
        es = [p[1] for p in slots]
        in_maps.append({
            "xT": xT,
            "w1": np.ascontiguousarray(w1n[es]),
            "b1": np.ascontiguousarray(b1n[es]),
            "w2": np.ascontiguousarray(w2n[es]),
        })
        core_pieces.append(slots)

    from concourse.bass_utils import run_bass_kernel_spmd
    res = run_bass_kernel_spmd(nc, in_maps, list(range(N_CORES)))
    global _LAST_RESULT
    _LAST_RESULT = res

    out = np.zeros_like(xf)
    for c in range(N_CORES):
        yT = res.results[c]["yT"]                    # [D, ncols] bf16
        for j, (n, e, sel, w) in enumerate(core_pieces[c]):
            if n == 0:
                continue
            y = yT[:, offs[j]:offs[j] + n].T.astype(np.float32) + b2n[e]
            out[sel] += w[:, None] * y
    return out.reshape(B, S, D)
